# revision 48
# baseline (speedup 1.0000x reference)
"""Trainium2 Bass kernel for nn_Block_19121194402322 (dense_transformer).

Math notes (validated numerically against the reference):
  - The reference einsum 'bnqk,bnvd->bnqd' contracts over BOTH k and v, so
    out[b,n,q,d] = (sum_k softmax(...)[q,k]) * (sum_v v[b,n,v,d]).  Softmax rows
    sum to exactly 1, so the whole Q/K/softmax pipeline is dead code; the
    attention output is the per-head sum of v broadcast over q.
  - After the (non-standard) reshape, head n owns flat sub-rows
    r in [1024n, 1024(n+1)) of (x@Wv).reshape(12288, 64), r = 12 s + c.
    With a 0/1 selector A:  YT = x^T A,  and w is a gather-sum of 64-wide
    diagonal blocks of YT^T Wv, emitted here directly in LN column layout.
  - LN(out_attn) is one 768-vector per batch element broadcast over the
    sequence:  a = x + lnvec.  Therefore
        a @ W1 + b1 = x @ W1 + (lnvec @ W1 + b1) = x @ W1 + v1
    which decouples the big matmul from the attention path entirely; v1 is a
    per-output-channel bias folded into the GELU.
  - MLP: h = gelu(a@W1 + b1); m = gelu(h@W2 + b2); out = x + LN2(m).

Precision scheme (rel err ~3e-3 vs fp32 reference, gate is 2e-2):
  - mm1 (x@W1) runs as a 3-term split-fp8 matmul in DoubleRow perf mode:
    x ~ x_hi + x_lo and W1 ~ w_hi + w_lo, each pair e4m3-quantized on the
    host at a shared power-of-2 scale; x_hi@w_hi + x_hi@w_lo + x_lo@w_hi
    drops only the lo*lo term (~0.1%).
  - Stage 1 (selector) reuses the hi/lo trick: DoubleRow pairs token chunks,
    summing hi and lo passes for bf16-grade accuracy at fp8 speed.
  - mm2, Wv, and the h activations stay bf16; LN statistics in fp32;
    residual x in fp32; output written as bf16.

Distribution: pure data-parallel over batch B=8 across the 8 NeuronCores
(one batch element per core); weights replicated.  No collectives.
"""

import numpy as np

S = 1024
E = 768
HID = 1536
HEADS = 12
HD = 64
EPS = 1e-5
P = 128
N_CORES = 8
KE = 6        # E / P contraction chunks
KH = 12       # HID / P contraction chunks
OT = 8        # S / P token tiles
ACOLS = 144   # selector columns: col = c*12 + (n%2)*6 + n//2   (c<12, n<12)

SX = 32.0     # x fp8 scale
SW1 = 512.0   # W1 fp8 scale
SLN = 16.0    # lnvec fp8 scale
MM1_DESCALE = 1.0 / (SX * SW1)
V1_DESCALE = 1.0 / (SLN * SW1)

_CACHE = {}


def _selector_cols():
    """For token s and chunk c the head is n = (12 s + c) // 1024; the packed
    column index places the 6 even-n (a=0) then 6 odd-n (a=1) heads of each c
    contiguously so stage 2's rhs slices are unit-stride."""
    s = np.arange(S)
    cols = np.zeros((S, ACOLS), np.float32)
    for c in range(HEADS):
        n = (HEADS * s + c) // S
        cols[s, c * 12 + (n % 2) * 6 + n // 2] = 1.0
    return cols


def _split_multi_waits(m):
    """Hoist all-but-one sync waits of each instruction onto preceding
    single-wait EventSemaphore instructions on the same engine.  Several TPB
    instruction structs carry only one sync-wait slot, and walrus codegen
    errors on more."""
    counter = [0]

    def fix_block(blk):
        out = []
        for inst in blk.get("instructions", []):
            si = inst.get("sync_info")
            waits = (si or {}).get("on_wait") or []
            if si and len(waits) > 1 and inst.get("opcode") != "EventSemaphore":
                for w in waits[:-1]:
                    counter[0] += 1
                    out.append({
                        "debug": inst.get("debug", 0), "engine": inst["engine"],
                        "ins": [], "outs": [], "name": f"I-wsplit-{counter[0]}",
                        "opcode": "EventSemaphore",
                        "sync_info": {"on_update": [], "on_wait": [w]},
                    })
                si["on_wait"] = waits[-1:]
            out.append(inst)
        blk["instructions"] = out
        for sub in blk.get("blocks", []):
            fix_block(sub)

    for fn in m["functions"]:
        for blk in fn["blocks"]:
            fix_block(blk)
    return m


def _build_bass(trivial=True):
    """trivial=True specializes the module for the case where every affine
    parameter is an identity (b1=b2=beta1=beta2=0, g1=g2=1), verified on the
    host before this variant is selected; the general path stays available."""
    import json
    import concourse.bass as bass
    import concourse.mybir as mybir
    import concourse.tile as tile

    f32 = mybir.dt.float32
    bf16 = mybir.dt.bfloat16
    fp8 = mybir.dt.float8e4
    AX = mybir.AxisListType.X
    OP = mybir.AluOpType
    AF = mybir.ActivationFunctionType
    DR = mybir.MatmulPerfMode.DoubleRow

    nc = bass.Bass(trn_type="TRN2")

    # All DRAM tensors are host-prepared in partition-major layout so each
    # partition is one contiguous DMA descriptor.
    at_d = nc.declare_dram_parameter("at8", [P, OT, ACOLS], fp8, isOutput=False)
    x8h_d = nc.declare_dram_parameter("x8hi", [P, OT, E], fp8, isOutput=False)
    x8l_d = nc.declare_dram_parameter("x8lo", [P, OT, E], fp8, isOutput=False)
    xth_d = nc.declare_dram_parameter("xt8hi", [P, KE, S], fp8, isOutput=False)
    xtl_d = nc.declare_dram_parameter("xt8lo", [P, KE, S], fp8, isOutput=False)
    w1h_d = nc.declare_dram_parameter("w1hi", [P, KH, KE, P], fp8, isOutput=False)
    w1l_d = nc.declare_dram_parameter("w1lo", [P, KH, KE, P], fp8, isOutput=False)
    wv_d = nc.declare_dram_parameter("wvb", [P, KE, E], bf16, isOutput=False)
    w2_d = nc.declare_dram_parameter("w2b", [P, KH, E], bf16, isOutput=False)
    xr_d = nc.declare_dram_parameter("xr", [P, OT, E],
                                     bf16 if trivial else f32, isOutput=False)
    sm_d = nc.declare_dram_parameter("smalls", [P, 24], f32, isOutput=False)
    if not trivial:
        b2_d = nc.declare_dram_parameter("b2", [E], f32, isOutput=False)
        g2_d = nc.declare_dram_parameter("g2", [E], bf16, isOutput=False)
        be2_d = nc.declare_dram_parameter("beta2", [E], f32, isOutput=False)
    out_d = nc.declare_dram_parameter("out", [P, OT, E], bf16, isOutput=True)

    with tile.TileContext(nc) as tc:
        with (
            tc.tile_pool(name="atp", bufs=1) as atp,
            tc.tile_pool(name="x8p", bufs=1) as x8p,
            tc.tile_pool(name="xtp", bufs=1) as xtp,
            tc.tile_pool(name="w1p", bufs=1) as w1p,
            tc.tile_pool(name="wvp", bufs=1) as wvp,
            tc.tile_pool(name="w2p", bufs=1) as w2p,
            tc.tile_pool(name="ytp", bufs=1) as ytp,
            tc.tile_pool(name="gp", bufs=1) as gp,
            tc.tile_pool(name="small", bufs=1) as small,
            tc.tile_pool(name="consts", bufs=1) as consts,
            tc.tile_pool(name="xrp", bufs=1) as xrp,
            tc.tile_pool(name="xwp", bufs=2) as xwp,
            tc.tile_pool(name="mp", bufs=2) as mp,
            tc.tile_pool(name="outp", bufs=2) as outp,
            tc.tile_pool(name="stat", bufs=4) as statp,
            tc.tile_pool(name="psA", bufs=6, space="PSUM") as psA,
            tc.tile_pool(name="psB", bufs=2, space="PSUM") as psB,
        ):
            # ---- DMA issue order == priority on the shared DMA engines ----
            smalls = consts.tile([P, 24], f32)
            nc.sync.dma_start(out=smalls, in_=sm_d[:])
            b1col = smalls[:, 0:12]
            g1col = smalls[:, 12:18]
            be1col = smalls[:, 18:24]
            at_sb = atp.tile([P, OT, ACOLS], fp8)
            nc.sync.dma_start(out=at_sb, in_=at_d[:])
            xth = xtp.tile([P, KE, S], fp8, tag="xth")
            nc.sync.dma_start(out=xth, in_=xth_d[:])
            xtl = xtp.tile([P, KE, S], fp8, tag="xtl")
            nc.sync.dma_start(out=xtl, in_=xtl_d[:])

            # W1 in triples of output chunks, hi/lo interleaved, so the mm1
            # j2 pipeline starts three chunks at a time; the selector-path
            # inputs (x8, Wv) follow the first triple — their compute slots in
            # behind the mm1 front-run.
            w1h = w1p.tile([P, KH, KE, P], fp8, tag="w1h")
            w1l = w1p.tile([P, KH, KE, P], fp8, tag="w1l")
            wv_sb = wvp.tile([P, KE, E], bf16)
            x8h = x8p.tile([P, OT, E], fp8, tag="x8h")
            x8l = x8p.tile([P, OT, E], fp8, tag="x8l")
            for t in range(4):
                nc.sync.dma_start(out=w1h[:, 3 * t:3 * (t + 1)],
                                  in_=w1h_d[:, 3 * t:3 * (t + 1)])
                nc.sync.dma_start(out=w1l[:, 3 * t:3 * (t + 1)],
                                  in_=w1l_d[:, 3 * t:3 * (t + 1)])
                if t == 0:
                    nc.sync.dma_start(out=x8h, in_=x8h_d[:])
                    nc.sync.dma_start(out=x8l, in_=x8l_d[:])
                    nc.sync.dma_start(out=wv_sb[:, 0:3, :], in_=wv_d[:, 0:3, :])
                    nc.sync.dma_start(out=wv_sb[:, 3:6, :], in_=wv_d[:, 3:6, :])

            w2_sb = w2p.tile([P, KH, E], bf16)
            nc.sync.dma_start(out=w2_sb, in_=w2_d[:])

            # residual x and the broadcast vectors, needed only by the mm2
            # tail: issued last on the same queue.
            xr_sb = xrp.tile([P, OT, E], bf16 if trivial else f32)
            nc.sync.dma_start(out=xr_sb[:, 0:4, :], in_=xr_d[:, 0:4, :])
            if not trivial:
                b2b = consts.tile([P, E], f32)
                nc.sync.dma_start(out=b2b, in_=b2_d[:].partition_broadcast(P))
                g2b = consts.tile([P, E], bf16)
                nc.sync.dma_start(out=g2b, in_=g2_d[:].partition_broadcast(P))
                be2b = consts.tile([P, E], f32)
                nc.sync.dma_start(out=be2b, in_=be2_d[:].partition_broadcast(P))
            nc.sync.dma_start(out=xr_sb[:, 4:8, :], in_=xr_d[:, 4:8, :])

            eps_sb = consts.tile([P, 1], f32)
            nc.vector.memset(eps_sb, EPS)
            ones_bf = consts.tile([P, 1], bf16)
            nc.vector.memset(ones_bf, 1.0)
            warm = consts.tile([P, P], bf16)
            nc.vector.memset(warm, 0.001)

            # ---- PE warmup: ramp the p-state while DMAs stream ------------
            pw = psB.tile([P, 512], f32, tag="pb")
            for _ in range(95):
                nc.tensor.matmul(pw[:, :P], warm, warm, start=True, stop=True)

            # ---- mm1 psum groups (PE) — gelus deferred --------------------
            # Every mm1 matmul needs only xt + its W1 chunk; only the GELU
            # needs the attention path (v1col).  Emitting the first chunks'
            # matmuls before stage 2 lets the PE start as soon as DMAs land,
            # while the LN-chain ACT ops stay ahead of all gelus in ACT
            # program order (avoiding an in-order ACT deadlock).
            g_sb = gp.tile([P, KH, S], bf16)
            v1col = small.tile([P, KH], f32)
            mm1_ps = {}

            def mm1_mms(j2, half):
                ps = psA.tile([P, 512], f32, tag="ps", name=f"mm1_{j2}_{half}")
                lo = 512 * half
                n = 0
                for xq, w1q in ((xth, w1h), (xth, w1l), (xtl, w1h)):
                    for jp in range(KE // 2):
                        n += 1
                        nc.tensor.matmul(
                            ps, w1q[:, j2, 2 * jp:2 * jp + 2, :],
                            xq[:, 2 * jp:2 * jp + 2, lo:lo + 512],
                            start=(n == 1), stop=(n == 9),
                            perf_mode=DR,
                        )
                mm1_ps[(j2, half)] = ps

            def mm1_gelu(j2, half):
                lo = 512 * half
                nc.scalar.activation(
                    out=g_sb[:, j2, lo:lo + 512], in_=mm1_ps.pop((j2, half)),
                    func=AF.Gelu, scale=MM1_DESCALE, bias=v1col[:, j2:j2 + 1],
                )

            def stage2_mms(psl, ks):
                for k in ks:
                    for a in range(2):
                        for c in range(HEADS):
                            nc.tensor.matmul(
                                psl[64 * a:64 * (a + 1), 0:KE],
                                wv_sb[:, k, c * HD:(c + 1) * HD],
                                yt[:, k, c * 12 + 6 * a:c * 12 + 6 * (a + 1)],
                                start=(k == 0 and a == 0 and c == 0),
                                stop=(k == KE - 1 and a == 1 and c == HEADS - 1),
                                skip_group_check=True,
                            )

            mm1_mms(0, 0)
            mm1_mms(0, 1)
            mm1_mms(1, 0)
            mm1_mms(1, 1)
            mm1_mms(2, 0)
            mm1_mms(2, 1)

            # ---- stage 1: YT = x^T A (fp8 DoubleRow, hi+lo passes) --------
            # Sequential over e-chunks on the psB ping-pong pair, so it can't
            # deadlock against the six held mm1 psum groups above.
            yt = ytp.tile([P, KE, ACOLS], bf16)
            for i in range(KE):
                s1 = psB.tile([P, 512], f32, tag="pb", name=f"s1_{i}")
                for term in (x8h, x8l):
                    for op in range(OT // 2):
                        nc.tensor.matmul(
                            s1[:, :ACOLS],
                            term[:, 2 * op:2 * op + 2, i * P:(i + 1) * P],
                            at_sb[:, 2 * op:2 * op + 2, :],
                            start=(term is x8h and op == 0),
                            stop=(term is x8l and op == OT // 2 - 1),
                            perf_mode=DR,
                        )
                if i % 2 == 0:
                    nc.scalar.activation(out=yt[:, i, :], in_=s1[:, :ACOLS],
                                         func=AF.Copy)
                else:
                    nc.vector.tensor_copy(yt[:, i, :], s1[:, :ACOLS])

            # stage 2: by now Wv has long landed, so the PE never stalls
            # mid-stream (which would reset the clock ramp).
            # psl[64a+d, j] = w[(2j+a)*64+d] = lnpre[j*128+(64a+d)]
            psl = psB.tile([P, 512], f32, tag="pb")
            stage2_mms(psl, range(0, KE))
            wcol = small.tile([P, KE], f32)
            nc.scalar.activation(out=wcol, in_=psl[:, 0:KE], func=AF.Copy)

            # ---- LN1 stats via ones-matmul reduction ----------------------
            sq = small.tile([P, KE], f32)
            nc.vector.tensor_mul(sq, wcol, wcol)
            red = small.tile([P, 2], bf16)
            with nc.allow_low_precision(reason="bf16 partial sums feed a "
                                        "768-term mean; 0.4% on partials is "
                                        "<0.1% on the stats"):
                nc.vector.tensor_reduce(out=red[:, 0:1], in_=wcol, axis=AX,
                                        op=OP.add)
                nc.vector.tensor_reduce(out=red[:, 1:2], in_=sq, axis=AX,
                                        op=OP.add)
            pst = psB.tile([P, 512], f32, tag="pb")
            nc.tensor.matmul(pst[:1, :2], ones_bf, red, start=True, stop=True)
            tots = small.tile([1, 2], f32)  # [mu, E[w^2]]
            nc.vector.tensor_scalar_mul(tots, pst[:1, :2], 1.0 / E)
            mu2 = small.tile([1, 1], f32)
            nc.vector.tensor_mul(mu2, tots[:, 0:1], tots[:, 0:1])
            mr = small.tile([32, 2], f32)  # [mu, rstd] valid on partition 0
            nc.vector.tensor_sub(mr[:1, 1:2], tots[:, 1:2], mu2)
            nc.scalar.activation(out=mr[:1, 1:2], in_=mr[:1, 1:2], func=AF.Sqrt,
                                 bias=eps_sb[:1])
            nc.vector.reciprocal(mr[:1, 1:2], mr[:1, 1:2])
            if trivial:
                # fold the SLN scale into rstd so lnsc comes out of one op
                nc.vector.tensor_scalar_mul(mr[:1, 1:2], mr[:1, 1:2], SLN)
            nc.vector.tensor_copy(mr[:1, 0:1], tots[:, 0:1])
            mrb = small.tile([P, 2], f32)
            for q in range(4):
                nc.vector.stream_shuffle(mrb[32 * q:32 * (q + 1), :], mr[:, :],
                                         [0] * 32)

            # lnsc[p, j] = lnvec[j*128+p] * SLN  (g1col/be1col are pre-scaled
            # by SLN on the host); then split to fp8 hi/lo.
            lnsc = small.tile([P, KE], f32)
            nc.vector.tensor_scalar(lnsc, wcol, mrb[:, 0:1], mrb[:, 1:2],
                                    OP.subtract, OP.mult)
            if not trivial:
                nc.vector.tensor_mul(lnsc, lnsc, g1col)
                nc.vector.tensor_add(lnsc, lnsc, be1col)
            ln8h = small.tile([P, KE, 1], fp8)
            nc.vector.tensor_copy(ln8h[:, :, 0], lnsc)
            ln8hf = small.tile([P, KE], f32)
            nc.vector.tensor_copy(ln8hf, ln8h[:, :, 0])
            ln8l = small.tile([P, KE, 1], fp8)
            nc.vector.tensor_sub(ln8l[:, :, 0], lnsc, ln8hf)

            # third warmup burst: bridge the LN-chain latency on PE.  Writes
            # a column range of the psB bank that no ln/stats/v1 AP touches
            # (start=False avoids the whole-bank zero, which would add a WAR
            # dependency on the LN chain's psum reads).
            for _ in range(26):
                nc.tensor.matmul(pw[:, P:2 * P], warm, warm,
                                 start=False, stop=True,
                                 skip_group_check=True)

            # ---- v1 = lnvec@W1 + b1 as tiny fp8 matmuls -------------------
            # Each matmul writes its own psum column (no accumulation chain);
            # a strided DVE reduce folds the 18 partial columns per chunk.
            # Two psum tiles so the first half can be read (and its gelus
            # unblocked) while the second half still waits on late W1 DMAs,
            # whose chunks are interleaved between mm1 groups below.
            def v1_mms(psv, cs):
                n = [0]
                for c in cs:
                    for lnq, w1q in ((ln8h, w1h), (ln8h, w1l), (ln8l, w1h)):
                        for k in range(KE):
                            col = 18 * (c - cs[0]) + n[0] % 18
                            nc.tensor.matmul(
                                psv[:, col:col + 1],
                                w1q[:, c, k, :],
                                lnq[:, k, :],
                                start=(n[0] == 0),
                                stop=(n[0] == 18 * len(cs) - 1),
                                skip_group_check=True,
                            )
                            n[0] += 1

            def v1_col(psv, cs):
                sl = v1col[:, cs[0]:cs[-1] + 1]
                nc.vector.tensor_reduce(
                    out=sl,
                    in_=psv[:, 0:18 * len(cs)].rearrange(
                        "p (c n) -> p c n", n=18),
                    axis=AX, op=OP.add)
                nc.vector.tensor_scalar_mul(sl, sl, V1_DESCALE)
                if not trivial:
                    nc.vector.tensor_add(sl, sl, b1col[:, cs[0]:cs[-1] + 1])

            psvA = psB.tile([P, 512], f32, tag="pb")
            v1_mms(psvA, list(range(0, 6)))
            v1_col(psvA, list(range(0, 6)))

            # drain the early psum groups now that v1col[:, 0:6] flows; the
            # later v1 chunk-groups slot between mm1 groups so their (late)
            # W1 DMA arrivals never idle the PE.
            for j2 in range(3):
                mm1_gelu(j2, 0)
                mm1_gelu(j2, 1)
            for j2 in (3, 4, 5):
                mm1_mms(j2, 0)
                mm1_gelu(j2, 0)
                mm1_mms(j2, 1)
                mm1_gelu(j2, 1)
            psvB1 = psB.tile([P, 512], f32, tag="pb")
            v1_mms(psvB1, list(range(6, 9)))
            v1_col(psvB1, list(range(6, 9)))
            for j2 in (6, 7):
                mm1_mms(j2, 0)
                mm1_gelu(j2, 0)
                mm1_mms(j2, 1)
                mm1_gelu(j2, 1)
            psvB2 = psB.tile([P, 512], f32, tag="pb")
            v1_mms(psvB2, list(range(9, KH)))
            v1_col(psvB2, list(range(9, KH)))
            for j2 in range(8, KH):
                mm1_mms(j2, 0)
                mm1_gelu(j2, 0)
                mm1_mms(j2, 1)
                mm1_gelu(j2, 1)

            # ---- mm2 (bf16) + LN2 + residual ------------------------------
            if not trivial:
                # xw = x + beta2 precomputed on the otherwise idle Pool engine
                xw_sb = xwp.tile([P, OT, E], bf16)
                with nc.allow_low_precision(reason="residual+beta2 feeds a "
                                            "bf16 output"):
                    for o in range(OT):
                        nc.gpsimd.tensor_add(xw_sb[:, o, :], xr_sb[:, o, :],
                                             be2b)
            else:
                xw_sb = xr_sb

            for o in range(OT):
                ps0 = psA.tile([P, 512], f32, tag="ps")
                ps1b = psA.tile([P, 512], f32, tag="ps")
                for k in range(KH):
                    lhs = g_sb[:, k, o * P:(o + 1) * P]
                    nc.tensor.matmul(ps0[:, :384], lhs, w2_sb[:, k, 0:384],
                                     start=(k == 0), stop=(k == KH - 1))
                    nc.tensor.matmul(ps1b[:, :384], lhs, w2_sb[:, k, 384:768],
                                     start=(k == 0), stop=(k == KH - 1))

                msb = mp.tile([P, E], bf16, tag="m")
                stats = statp.tile([P, 2, 6], f32, tag="st")
                with nc.allow_low_precision(reason="m is consumed in bf16; "
                                            "LN2 stats tolerate 0.4% on m"):
                    if trivial:
                        nc.scalar.activation(out=msb[:, 0:384],
                                             in_=ps0[:, :384], func=AF.Gelu)
                        nc.scalar.activation(out=msb[:, 384:768],
                                             in_=ps1b[:, :384], func=AF.Gelu)
                    else:
                        nc.vector.tensor_add(msb[:, 0:384], ps0[:, :384],
                                             b2b[:, 0:384])
                        nc.scalar.activation(out=msb[:, 0:384],
                                             in_=msb[:, 0:384], func=AF.Gelu)
                        nc.vector.tensor_add(msb[:, 384:768], ps1b[:, :384],
                                             b2b[:, 384:768])
                        nc.scalar.activation(out=msb[:, 384:768],
                                             in_=msb[:, 384:768], func=AF.Gelu)
                nc.vector.bn_stats(out=stats[:, 0, :], in_=msb[:, 0:384])
                nc.vector.bn_stats(out=stats[:, 1, :], in_=msb[:, 384:768])
                mv = statp.tile([P, 2], f32, tag="mv")
                nc.vector.bn_aggr(out=mv, in_=stats)
                rstd = statp.tile([P, 1], f32, tag="rstd")
                nc.scalar.activation(out=rstd, in_=mv[:, 1:2], func=AF.Sqrt,
                                     bias=eps_sb)
                nc.vector.reciprocal(rstd, rstd)

                u = outp.tile([P, E], bf16, tag="u")
                with nc.allow_low_precision(reason="bf16 output precision"):
                    nc.vector.tensor_scalar(msb, msb, mv[:, 0:1], rstd,
                                            OP.subtract, OP.mult)
                    if not trivial:
                        nc.vector.tensor_mul(msb, msb, g2b)
                    nc.vector.tensor_add(u[:, 0:384], msb[:, 0:384],
                                         xw_sb[:, o, 0:384])
                    nc.vector.tensor_add(u[:, 384:768], msb[:, 384:768],
                                         xw_sb[:, o, 384:768])
                    if o < OT - 1:
                        nc.sync.dma_start(out=out_d[:, o, 0:384],
                                          in_=u[:, 0:384])
                        nc.gpsimd.dma_start(out=out_d[:, o, 384:768],
                                            in_=u[:, 384:768])
                    else:
                        # single full-width transfer for the final tile: one
                        # DGE config on the otherwise-idle sync queue
                        nc.sync.dma_start(out=out_d[:, o, :], in_=u)

    m = json.loads(mybir.module_to_json_bytes(nc.m))
    m = _split_multi_waits(m)
    nc.m = mybir.module_from_json_bytes(json.dumps(m).encode())
    return nc


def _is_trivial_affine(inputs):
    def zero(k):
        return not np.any(np.asarray(inputs[k]))

    def one(k):
        return bool(np.all(np.asarray(inputs[k]) == 1.0))

    return (zero("b1") and zero("b2") and zero("beta1") and zero("beta2")
            and one("g1") and one("g2"))


def _prep_inputs(inputs, trivial):
    import ml_dtypes
    E4 = ml_dtypes.float8_e4m3
    BF = ml_dtypes.bfloat16

    def f32c(a):
        return np.ascontiguousarray(np.asarray(a), dtype=np.float32)

    def split8(a, scale):
        hi = (a * scale).astype(E4)
        lo = (a * scale - hi.astype(np.float32)).astype(E4)
        return hi, lo

    Wv = f32c(inputs["Wv"])
    W1 = f32c(inputs["W1"])
    W2 = f32c(inputs["W2"])

    # [P, KH, KE, P] j2-major W1 chunks
    w1r = np.ascontiguousarray(
        W1.reshape(KE, P, KH, P).transpose(1, 2, 0, 3))
    w1hi, w1lo = split8(w1r, SW1)

    at8 = np.ascontiguousarray(
        _selector_cols().reshape(OT, P, ACOLS).transpose(1, 0, 2))
    shared = {
        "at8": at8.astype(E4),
        "w1hi": w1hi, "w1lo": w1lo,
        "wvb": np.ascontiguousarray(
            Wv.reshape(KE, P, E).transpose(1, 0, 2)).astype(BF),
        "w2b": np.ascontiguousarray(
            W2.reshape(KH, P, E).transpose(1, 0, 2)).astype(BF),
    }
    if not trivial:
        shared["b2"] = f32c(inputs["b2"])
        shared["g2"] = f32c(inputs["g2"]).astype(BF)
        shared["beta2"] = f32c(inputs["beta2"])
    sm = np.zeros((P, 24), np.float32)
    sm[:, 0:12] = f32c(inputs["b1"]).reshape(KH, P).T
    sm[:, 12:18] = f32c(inputs["g1"]).reshape(KE, P).T * SLN
    sm[:, 18:24] = f32c(inputs["beta1"]).reshape(KE, P).T * SLN
    shared["smalls"] = sm

    x = f32c(inputs["x"])  # (B, S, E)
    per_core = []
    for b in range(x.shape[0]):
        xb = x[b]
        xtok = np.ascontiguousarray(
            xb.reshape(OT, P, E).transpose(1, 0, 2))            # [P, OT, E]
        xT = np.ascontiguousarray(
            np.ascontiguousarray(xb.T).reshape(KE, P, S).transpose(1, 0, 2))
        x8hi, x8lo = split8(xtok, SX)
        xt8hi, xt8lo = split8(xT, SX)
        per_core.append(dict(
            shared,
            x8hi=x8hi, x8lo=x8lo, xt8hi=xt8hi, xt8lo=xt8lo,
            xr=xtok.astype(BF) if trivial else xtok,
        ))
    return per_core


def _run(inputs, trace=False):
    from concourse.bass_utils import run_bass_kernel_spmd

    trivial = _is_trivial_affine(inputs)
    key = f"nc_{trivial}"
    if key not in _CACHE:
        _CACHE[key] = _build_bass(trivial=trivial)
    nc = _CACHE["nc"] = _CACHE[key]

    in_maps = _prep_inputs(inputs, trivial)
    res = run_bass_kernel_spmd(
        nc, in_maps, core_ids=list(range(N_CORES)), trace=trace,
        **({"trace_cores": list(range(N_CORES))} if trace else {}),
    )
    outs = []
    for r in res.results:
        ob = np.asarray(r["out"]).astype(np.float32)   # [P, OT, E]
        outs.append(ob.transpose(1, 0, 2).reshape(S, E))
    return np.stack(outs, axis=0), res


def kernel(x, Wq=None, Wk=None, Wv=None, W1=None, b1=None, W2=None, b2=None,
           g1=None, beta1=None, g2=None, beta2=None):
    out, _ = _run(dict(x=x, Wv=Wv, W1=W1, b1=b1, W2=W2, b2=b2, g1=g1,
                       beta1=beta1, g2=g2, beta2=beta2))
    return out


def kernel_profiled(**inputs):
    out, res = _run(inputs, trace=True)
    return out, res
